# revision 1
# baseline (speedup 1.0000x reference)
# Trainium2 Bass kernel for nn_LorentzSparseSqDisAtt (GNN edge attention).
#
# reference:
#   u  = log0_tail(x); mu = u @ W^T + b; y = exp0(mu)        [LorentzLinear]
#   res[e] = exp(-clip(-(1 + <y[src_e], y[dst_e]>_L), 1e-10, 1))
#
# Device strategy (8 cores, full I/O):
#   Phase 1 (replicated on every core): build a DRAM table with 512B rows
#     row[i] = [tail_i (64 f32), y0_i, pad...]  where y = exp0(mu).
#     - Host passes x pre-transposed (xT) so the PE matmul mu0 = xt @ W^T
#       needs no on-device transpose (lhsT = xT slice directly).
#     - Per-node scalars derive from x0 alone via the hyperboloid identity
#       |xt|^2 = x0^2 - 1  (c = 1).
#   Phase 2 (edges sharded 100k/core): per 4096-edge tile, two
#     gpsimd.dma_gather of 512B rows (src side, dst side), then
#     l = sum(tailA*tailB) ; t = y0A*y0B - 1 - l ; res = exp(-clip(t)).
#     dma_gather indices are int16, so host buckets each core's edges into
#     4 groups by (src < 32768, dst < 32768); the hi-range gathers use a
#     table view offset by 32768. Host un-permutes the result.
import numpy as np

N = 50000
DSP = 64          # spatial dim
E = 800000
NCORES = 8
SPLIT = 32768     # int16 index limit for dma_gather
NPAD = 50176      # 49 * 1024 node rows (padded)
ROW = 128         # table row elems (f32) = 512B
SUPER = 1024      # phase-1 nodes per supertile
NSUP = NPAD // SUPER
TILE_E = 4096     # edges per phase-2 gather tile

_prog_cache = {}


def _build_program(views, bias_nonzero):
    """views: tuple of (src_lo: bool, dst_lo: bool) per phase-2 tile."""
    from contextlib import ExitStack

    import concourse.bacc as bacc
    import concourse.bass as bass
    import concourse.tile as tile
    from concourse import mybir

    f32 = mybir.dt.float32
    i16 = mybir.dt.int16
    AF = mybir.ActivationFunctionType
    OP = mybir.AluOpType
    n_tiles = len(views)

    nc = bacc.Bacc(
        "TRN2",
        target_bir_lowering=False,
        debug=False,
        enable_asserts=False,
        num_devices=NCORES,
    )

    xT = nc.dram_tensor("xT", [DSP + 1, NPAD], f32, kind="ExternalInput").ap()
    x0w = nc.dram_tensor("x0w", [128, NPAD // 128], f32, kind="ExternalInput").ap()
    wt = nc.dram_tensor("wt", [DSP, DSP], f32, kind="ExternalInput").ap()
    bias_d = nc.dram_tensor("bias", [1, DSP], f32, kind="ExternalInput").ap()
    idx = nc.dram_tensor(
        "idx", [max(n_tiles, 1), 2, 128, TILE_E // 16], i16, kind="ExternalInput"
    ).ap()
    res = nc.dram_tensor(
        "res", [max(n_tiles, 1), 128, TILE_E // 128], f32, kind="ExternalOutput"
    ).ap()
    table = nc.dram_tensor("table", [NPAD, ROW], f32).ap()

    with tile.TileContext(nc) as tc, ExitStack() as ctx:
        cpool = ctx.enter_context(tc.tile_pool(name="const", bufs=1))
        p1 = ctx.enter_context(tc.tile_pool(name="p1", bufs=3))
        p1s = ctx.enter_context(tc.tile_pool(name="p1s", bufs=3))
        pps = ctx.enter_context(tc.tile_pool(name="pps", bufs=8, space="PSUM"))

        neg1 = cpool.tile([128, 1], f32)
        nc.gpsimd.memset(neg1[:], -1.0)
        wt_t = cpool.tile([DSP, DSP], f32)
        nc.sync.dma_start(wt_t[:], wt)
        x0t = cpool.tile([128, NPAD // 128], f32)
        nc.sync.dma_start(x0t[:], x0w)
        if bias_nonzero:
            b_row = cpool.tile([1, DSP], f32)
            nc.sync.dma_start(b_row[:], bias_d)
            ones_col = cpool.tile([1, 128], f32)
            nc.gpsimd.memset(ones_col[:], 1.0)
            b_ps = pps.tile([128, DSP], f32)
            nc.tensor.matmul(b_ps[:], lhsT=ones_col[:], rhs=b_row[:],
                             start=True, stop=True)
            b_bc = cpool.tile([128, DSP], f32)
            nc.scalar.copy(b_bc[:], b_ps[:])

        # ---------------- Phase 1: build node table ----------------
        for st in range(NSUP):
            n0 = st * SUPER
            xtT = p1.tile([DSP, SUPER], f32, tag="xtT")
            nc.sync.dma_start(xtT[:], xT[1 : DSP + 1, n0 : n0 + SUPER])

            x0s = x0t[:, st * 8 : (st + 1) * 8]  # [128, 8]
            z = p1s.tile([128, 8], f32, tag="z")
            nc.vector.tensor_scalar_max(z[:], x0s, 1.0 + 1e-7)
            zsq = p1s.tile([128, 8], f32, tag="zsq")
            nc.vector.tensor_tensor(out=zsq[:], in0=z[:], in1=z[:], op=OP.mult)
            w0 = p1s.tile([128, 8], f32, tag="w0")
            nc.scalar.activation(w0[:], zsq[:], AF.Sqrt, bias=neg1[:])
            zw = p1s.tile([128, 8], f32, tag="zw")
            nc.vector.tensor_tensor(out=zw[:], in0=z[:], in1=w0[:], op=OP.add)
            dist = p1s.tile([128, 8], f32, tag="dist")
            nc.scalar.activation(dist[:], zw[:], AF.Ln)
            wc = p1s.tile([128, 8], f32, tag="wc")
            nc.vector.tensor_scalar_max(wc[:], w0[:], 1e-10)
            wci = p1s.tile([128, 8], f32, tag="wci")
            nc.vector.reciprocal(wci[:], wc[:])
            snd = p1s.tile([128, 8], f32, tag="snd")
            nc.vector.tensor_tensor(out=snd[:], in0=dist[:], in1=wci[:], op=OP.mult)

            mus = []
            msq = p1s.tile([128, 8], f32, tag="msq")
            scr = p1.tile([128, DSP], f32, tag="scr")
            mu_all = p1.tile([128, 8, DSP], f32, tag="mu_all") if bias_nonzero else None
            for s in range(8):
                mu0 = pps.tile([128, DSP], f32, tag="mu0")
                nc.tensor.matmul(
                    mu0[:],
                    lhsT=xtT[:, s * 128 : (s + 1) * 128],
                    rhs=wt_t[:],
                    start=True,
                    stop=True,
                )
                if bias_nonzero:
                    # mu = snd*mu0 + b ; keep in SBUF
                    nc.vector.tensor_scalar(
                        out=mu_all[:, s, :], in0=mu0[:], scalar1=snd[:, s : s + 1],
                        scalar2=None, op0=OP.mult,
                    )
                    nc.vector.tensor_tensor(
                        out=mu_all[:, s, :], in0=mu_all[:, s, :], in1=b_bc[:],
                        op=OP.add,
                    )
                    nc.scalar.activation(
                        scr[:], mu_all[:, s, :], AF.Square,
                        accum_out=msq[:, s : s + 1],
                    )
                else:
                    mus.append(mu0)
                    nc.scalar.activation(
                        scr[:], mu0[:], AF.Square, accum_out=msq[:, s : s + 1]
                    )

            # per-node tail scale g and head y0
            r0 = p1s.tile([128, 8], f32, tag="r0")
            nc.scalar.activation(r0[:], msq[:], AF.Sqrt)
            if not bias_nonzero:
                # r = snd * |mu0|
                nc.vector.tensor_tensor(out=r0[:], in0=r0[:], in1=snd[:], op=OP.mult)
            rc = p1s.tile([128, 8], f32, tag="rc")
            nc.vector.tensor_scalar_max(rc[:], r0[:], 1e-10)
            ep = p1s.tile([128, 8], f32, tag="ep")
            nc.scalar.activation(ep[:], rc[:], AF.Exp)
            em = p1s.tile([128, 8], f32, tag="em")
            nc.scalar.activation(em[:], rc[:], AF.Exp, scale=-1.0)
            y0 = p1s.tile([128, 8], f32, tag="y0")
            nc.vector.tensor_tensor(out=y0[:], in0=ep[:], in1=em[:], op=OP.add)
            nc.vector.tensor_scalar_mul(y0[:], y0[:], 0.5)
            f0 = p1s.tile([128, 8], f32, tag="f0")
            nc.vector.tensor_tensor(out=f0[:], in0=ep[:], in1=em[:], op=OP.subtract)
            rci = p1s.tile([128, 8], f32, tag="rci")
            nc.vector.reciprocal(rci[:], rc[:])
            g = p1s.tile([128, 8], f32, tag="g")
            nc.vector.tensor_tensor(out=g[:], in0=f0[:], in1=rci[:], op=OP.mult)
            if not bias_nonzero:
                # tail = (0.5*g*snd) * mu0   (fold snd here)
                nc.vector.tensor_tensor(out=g[:], in0=g[:], in1=snd[:], op=OP.mult)

            out_t = p1.tile([128, 8, ROW], f32, tag="out_t")
            for s in range(8):
                src_ap = mu_all[:, s, :] if bias_nonzero else mus[s][:]
                nc.vector.tensor_scalar(
                    out=out_t[:, s, 0:DSP], in0=src_ap, scalar1=g[:, s : s + 1],
                    scalar2=0.5, op0=OP.mult, op1=OP.mult,
                )
            nc.vector.tensor_copy(out_t[:, :, DSP], y0[:])
            # DRAM rows n0 + s*128 + p  (write cols 0..64 only)
            dst = table[n0 : n0 + SUPER, 0 : DSP + 1].rearrange(
                "(s p) f -> p s f", p=128
            )
            nc.sync.dma_start(dst, out_t[:, :, 0 : DSP + 1])

        tc.strict_bb_all_engine_barrier()

        # ---------------- Phase 2: edge gather + inner product ----------------
        lo_view = table[0:SPLIT, :]
        hi_view = table[SPLIT:NPAD, :]
        p2i = ctx.enter_context(tc.tile_pool(name="p2i", bufs=4))
        p2a = ctx.enter_context(tc.tile_pool(name="p2a", bufs=2))
        p2b = ctx.enter_context(tc.tile_pool(name="p2b", bufs=2))
        p2s = ctx.enter_context(tc.tile_pool(name="p2s", bufs=3))

        for t in range(n_tiles):
            src_lo, dst_lo = views[t]
            ia = p2i.tile([128, TILE_E // 16], i16, tag="ia")
            nc.sync.dma_start(ia[:], idx[t, 0])
            ib = p2i.tile([128, TILE_E // 16], i16, tag="ib")
            nc.sync.dma_start(ib[:], idx[t, 1])
            # HW: SWDGE descriptor ring caps dma_gather at ~1024 idxs/inst
            GC = 1024
            A = p2a.tile([128, TILE_E // 128, ROW], f32, tag="A")
            B = p2b.tile([128, TILE_E // 128, ROW], f32, tag="B")
            for k in range(TILE_E // GC):
                nc.gpsimd.dma_gather(
                    A[:, k * (GC // 128) : (k + 1) * (GC // 128), :],
                    lo_view if src_lo else hi_view,
                    ia[:, k * (GC // 16) : (k + 1) * (GC // 16)],
                    num_idxs=GC, num_idxs_reg=GC, elem_size=ROW,
                )
                nc.gpsimd.dma_gather(
                    B[:, k * (GC // 128) : (k + 1) * (GC // 128), :],
                    lo_view if dst_lo else hi_view,
                    ib[:, k * (GC // 16) : (k + 1) * (GC // 16)],
                    num_idxs=GC, num_idxs_reg=GC, elem_size=ROW,
                )
            P = p2s.tile([128, TILE_E // 128, DSP], f32, tag="P")
            nc.vector.tensor_tensor(
                out=P[:], in0=A[:, :, 0:DSP], in1=B[:, :, 0:DSP], op=OP.mult
            )
            T_ = p2s.tile([128, TILE_E // 128], f32, tag="T_")
            nc.vector.tensor_reduce(
                T_[:], P[:], axis=mybir.AxisListType.X, op=OP.add
            )
            m = p2s.tile([128, TILE_E // 128], f32, tag="m")
            nc.vector.tensor_tensor(
                out=m[:], in0=A[:, :, DSP], in1=B[:, :, DSP], op=OP.mult
            )
            # t = (m - 1) - T  ; clip to [1e-10, 1] ; res = exp(-t)
            tt = p2s.tile([128, TILE_E // 128], f32, tag="tt")
            nc.vector.scalar_tensor_tensor(
                out=tt[:], in0=m[:], scalar=1.0, in1=T_[:],
                op0=OP.subtract, op1=OP.subtract,
            )
            nc.vector.tensor_scalar(
                out=tt[:], in0=tt[:], scalar1=1e-10, scalar2=1.0,
                op0=OP.max, op1=OP.min,
            )
            rr = p2s.tile([128, TILE_E // 128], f32, tag="rr")
            nc.scalar.activation(rr[:], tt[:], AF.Exp, scale=-1.0)
            nc.sync.dma_start(res[t], rr[:])

    nc.compile()
    return nc


def _prep_core(src, dst):
    """Bucket one core's edges into 4 (src_lo, dst_lo) groups.
    Returns (order, counts[4], idx16 dict g -> (srcids, dstids))."""
    cat = (src >= SPLIT).astype(np.int64) * 2 + (dst >= SPLIT).astype(np.int64)
    order = np.argsort(cat, kind="stable")
    counts = np.bincount(cat, minlength=4)
    return order, counts


def _wrap_idx(ids):
    """[TILE_E] int16 -> [128, TILE_E//16] wrapped+replicated layout."""
    w = ids.reshape(TILE_E // 16, 16).T  # [16, 256]
    return np.ascontiguousarray(np.tile(w, (8, 1)))  # [128, 256]


def kernel(x, weight, bias, adj_indices):
    from concourse.bass_utils import run_bass_kernel_spmd

    x = np.asarray(x, dtype=np.float32)
    weight = np.asarray(weight, dtype=np.float32)
    bias_np = np.asarray(bias, dtype=np.float32)
    adj = np.asarray(adj_indices)
    Eall = adj.shape[1]
    EC = (Eall + NCORES - 1) // NCORES

    # ---- host prep: per-core edge bucketing ----
    cores = []
    for c in range(NCORES):
        lo, hi = c * EC, min((c + 1) * EC, Eall)
        src = adj[0, lo:hi].astype(np.int64)
        dst = adj[1, lo:hi].astype(np.int64)
        order, counts = _prep_core(src, dst)
        cores.append((src, dst, order, counts, lo, hi))

    # uniform per-group tile counts across cores (one SPMD program)
    tcounts = []
    for g in range(4):
        mx = max(int(c[3][g]) for c in cores)
        tcounts.append((mx + TILE_E - 1) // TILE_E)
    views = []
    for g, tg in enumerate(tcounts):
        src_lo, dst_lo = g < 2, (g % 2) == 0
        views += [(src_lo, dst_lo)] * tg
    views = tuple(views)
    n_tiles = len(views)

    # ---- per-core input tensors ----
    xp = np.zeros((NPAD, DSP + 1), dtype=np.float32)
    xp[:N] = x
    xp[N:, 0] = 1.0
    xT = np.ascontiguousarray(xp.T)                       # [65, NPAD]
    x0w = np.ascontiguousarray(xp[:, 0].reshape(NPAD // 128, 128).T)
    wt = np.ascontiguousarray(weight.T)                   # [k, j]
    b_in = np.ascontiguousarray(bias_np.reshape(1, DSP))
    bias_nonzero = bool(np.any(bias_np != 0.0))

    in_maps = []
    metas = []
    for c in range(NCORES):
        src, dst, order, counts, lo, hi = cores[c]
        idx_arr = np.zeros((n_tiles, 2, 128, TILE_E // 16), dtype=np.int16)
        pos = 0
        toff = 0
        meta = []  # (tile_range, group, count)
        for g, tg in enumerate(tcounts):
            cnt = int(counts[g])
            sel = order[pos : pos + cnt]
            pos += cnt
            s_ids = src[sel] - (0 if g < 2 else SPLIT)
            d_ids = dst[sel] - (0 if g % 2 == 0 else SPLIT)
            padded = tg * TILE_E
            s_p = np.zeros(padded, dtype=np.int16)
            d_p = np.zeros(padded, dtype=np.int16)
            s_p[:cnt] = s_ids.astype(np.int16)
            d_p[:cnt] = d_ids.astype(np.int16)
            for k in range(tg):
                idx_arr[toff + k, 0] = _wrap_idx(s_p[k * TILE_E : (k + 1) * TILE_E])
                idx_arr[toff + k, 1] = _wrap_idx(d_p[k * TILE_E : (k + 1) * TILE_E])
            meta.append((toff, tg, cnt, sel))
            toff += tg
        metas.append(meta)
        in_maps.append(
            {"xT": xT, "x0w": x0w, "wt": wt, "bias": b_in, "idx": idx_arr}
        )

    key = (views, bias_nonzero)
    if key not in _prog_cache:
        _prog_cache[key] = _build_program(views, bias_nonzero)
    nc = _prog_cache[key]

    import kernel as _self  # stash run args/results for the test harness

    _self.LAST_ARGS = (nc, in_maps)
    robj = run_bass_kernel_spmd(nc, in_maps, list(range(NCORES)))
    _self.LAST_RUN = robj
    results = robj.results

    # ---- host reassembly ----
    out = np.empty(Eall, dtype=np.float32)
    for c in range(NCORES):
        src, dst, order, counts, lo, hi = cores[c]
        r = results[c]["res"]  # [n_tiles, 128, TILE_E//128]
        local = np.empty(hi - lo, dtype=np.float32)
        for toff, tg, cnt, sel in metas[c]:
            if cnt == 0:
                continue
            # edge j*128+p of tile k lives at r[toff+k, p, j]
            flat = (
                r[toff : toff + tg].transpose(0, 2, 1).reshape(-1)
            )  # tile-major, j*128+p order
            local[sel] = flat[:cnt]
        out[lo:hi] = local
    return out

